# revision 11
# baseline (speedup 1.0000x reference)
"""Trainium2 Bass kernel: classical single-head attention layer.

reference math:
    qkv = x @ w_qkv.T        # x [8192, 512], w_qkv [192, 512]
    q, k, v = split(qkv, 3)  # each [8192, 64]
    out = softmax(q @ k.T / 8) @ v   # [8192, 64]

Sharding: Q row-blocks across 8 cores (1024 rows each); K/V replicated.
Two NEFF passes (host marshaling between them is free):
  pass 1 (per core c): bf16 projection of the core's 1024 x-rows ->
          Q^T/K^T [128, 1024] and V^T [64, 1024], all bf16 out.
  host:   concat K^T / V^T across cores, build pass-2 operand images.
  pass 2 (per core c): attention for the core's 1024 queries, processed as
          32 key-chunk PAIRS per 512-query block. Each pair (j, j+32) maps
          to PE row-tiles (0,0)/(64,0) so the two K=64 score matmuls run
          CONCURRENTLY in the 128x128 array. exp alternates between ACT
          (exact, scale folded into the affine) and DVE (Schraudolph bf16
          via one fused tensor_scalar -> int16 round, bitcast bf16).
          P^T @ [V|1] accumulates (PV)^T + softmax denominator in PSUM;
          the raw [65, 512] accumulators are DMA'd out and the divide +
          transpose happen on the host.
"""

import math
from contextlib import ExitStack

import ml_dtypes
import numpy as np

import concourse.bass as bass
import concourse.mybir as mybir
import concourse.tile as tile
from concourse import bacc
from concourse.bass_utils import run_bass_kernel_spmd

F32 = mybir.dt.float32
BF16 = mybir.dt.bfloat16
I16 = mybir.dt.int16
BF_NP = ml_dtypes.bfloat16

N = 8192          # sequence length
D_IN = 512        # input features
D = 64            # head dim (size_out)
NC = 8            # cores
SEQ_C = N // NC   # 1024 queries per core
SCALE = 1.0 / math.sqrt(D)

NPAIR = 32        # key-chunk pairs (chunk j pairs with j+32)
VP_W = 80         # V' chunk stride (65 used, 32B-aligned starts)

# bf16 Schraudolph exp: bf16_bits(exp(x)) ~= round(x*184.6645 + 16250.41)
SCH_C1 = 128.0 / math.log(2.0)
SCH_C2 = 127.0 * 128.0 - 366393.0 / 65536.0

# stash of BassKernelResults for test harness introspection
LAST_RESULTS = []

_CACHE = {}


def _build_pass1():
    """Projection pass: xt [512, 1024] bf16, wt_img [128, 768] bf16 ->
    qk [128, 1024] bf16 (rows 0:64 Q^T, rows 64:128 K^T), vt [64, 1024] bf16.

    wt_img is the host-packed SBUF image: wt_img[p, i*192+o] = w^T[i*128+p, o].
    """
    nc = bacc.Bacc("TRN2", target_bir_lowering=False, debug=False, num_devices=NC)
    xt_d = nc.dram_tensor("xt", [D_IN, SEQ_C], BF16, kind="ExternalInput")
    wt_d = nc.dram_tensor("wt", [128, 4 * 3 * D], BF16, kind="ExternalInput")
    qk_d = nc.dram_tensor("qk", [128, SEQ_C], BF16, kind="ExternalOutput")
    vt_d = nc.dram_tensor("vt", [D, SEQ_C], BF16, kind="ExternalOutput")

    with tile.TileContext(nc) as tc, ExitStack() as ctx:
        sb = ctx.enter_context(tc.tile_pool(name="sb", bufs=1))
        ps_a = ctx.enter_context(tc.tile_pool(name="ps_a", bufs=2, space="PSUM"))
        ps_b = ctx.enter_context(tc.tile_pool(name="ps_b", bufs=2, space="PSUM"))

        wt_sb = sb.tile([128, 4 * 3 * D], BF16)
        nc.sync.dma_start(wt_sb[:], wt_d[:, :])
        xt_sb = []
        for i in range(4):
            t = sb.tile([128, SEQ_C], BF16, tag=f"xt{i}")
            for h in range(2):
                nc.sync.dma_start(
                    t[:, h * 512 : (h + 1) * 512],
                    xt_d[i * 128 : (i + 1) * 128, h * 512 : (h + 1) * 512],
                )
            xt_sb.append(t)

        qk_sb = sb.tile([128, SEQ_C], BF16)
        vt_sb = sb.tile([D, SEQ_C], BF16)

        # Q^T/K^T: psum [128, 512] = sum_i WqkT_i.T @ xT_i
        for sblk in range(SEQ_C // 512):
            a = ps_a.tile([128, 512], F32)
            for i in range(4):
                nc.tensor.matmul(
                    a[:],
                    wt_sb[:, i * 192 : i * 192 + 128],
                    xt_sb[i][:, sblk * 512 : sblk * 512 + 512],
                    start=(i == 0),
                    stop=(i == 3),
                )
            nc.vector.tensor_copy(qk_sb[:, sblk * 512 : sblk * 512 + 512], a[:])
            nc.sync.dma_start(
                qk_d[:, sblk * 512 : sblk * 512 + 512],
                qk_sb[:, sblk * 512 : sblk * 512 + 512],
            )

        # V^T: psum [64, 512] x2 = sum_i WvT_i.T @ xT_i
        for sblk in range(SEQ_C // 512):
            b = ps_b.tile([D, 512], F32)
            for i in range(4):
                nc.tensor.matmul(
                    b[:],
                    wt_sb[:, i * 192 + 128 : i * 192 + 192],
                    xt_sb[i][:, sblk * 512 : sblk * 512 + 512],
                    start=(i == 0),
                    stop=(i == 3),
                )
            nc.scalar.copy(vt_sb[:, sblk * 512 : sblk * 512 + 512], b[:])
        nc.sync.dma_start(vt_d[:, :], vt_sb[:])

    nc.compile()
    return nc


def _build_pass2():
    """Attention pass per core.

    inputs : qt2 [128, 1024] (Q^T duplicated on both partition halves)
             kt2 [128, 4096] (K^T: rows 0:64 keys 0:4096, rows 64:128 the rest)
             vp  [128, 64*VP_W] (V' image: processing position p at cols p*VP_W;
                 position 2m = chunk m, 2m+1 = chunk m+32; col 64 = ones)
    output : acc [65, 1024] f32 (per q-block: rows 0:64 = (P V)^T, row 64 = denom)
    """
    nc = bacc.Bacc("TRN2", target_bir_lowering=False, debug=False, num_devices=NC)
    qt_d = nc.dram_tensor("qt2", [128, SEQ_C], BF16, kind="ExternalInput")
    kt_d = nc.dram_tensor("kt2", [128, N // 2], BF16, kind="ExternalInput")
    vp_d = nc.dram_tensor("vp", [128, (N // 128) * VP_W], BF16, kind="ExternalInput")
    acc_d = nc.dram_tensor("acc", [D + 1, SEQ_C], F32, kind="ExternalOutput")

    exp_f = mybir.ActivationFunctionType.Exp
    NQ = SEQ_C // 512  # q-blocks

    with tile.TileContext(nc) as tc, ExitStack() as ctx:
        sb = ctx.enter_context(tc.tile_pool(name="sb", bufs=1))
        p_pool = ctx.enter_context(tc.tile_pool(name="pT", bufs=5))
        fin_pool = ctx.enter_context(tc.tile_pool(name="fin", bufs=2))
        s_pool = ctx.enter_context(tc.tile_pool(name="sT", bufs=3, space="PSUM"))
        o_pool = ctx.enter_context(tc.tile_pool(name="oac", bufs=1, space="PSUM"))

        # preload the exp table while input DMAs are in flight
        scratch = fin_pool.tile([1, 1], F32, tag="scr")
        nc.vector.memset(scratch[:], 0.0)
        nc.scalar.activation(scratch[:], scratch[:], exp_f)

        qt_sb = sb.tile([128, SEQ_C], BF16)
        for h in range(2):
            nc.sync.dma_start(
                qt_sb[:, h * 512 : (h + 1) * 512], qt_d[:, h * 512 : (h + 1) * 512]
            )
        kt_sb = sb.tile([128, N // 2], BF16)
        vp_sb = sb.tile([128, (N // 128) * VP_W], BF16)
        # interleaved fine-grained pieces so pair m's operands land early:
        # piece i covers pairs 4i..4i+3
        for i in range(8):
            nc.sync.dma_start(
                kt_sb[:, i * 512 : (i + 1) * 512],
                kt_d[:, i * 512 : (i + 1) * 512],
            )
            nc.sync.dma_start(
                vp_sb[:, i * 8 * VP_W : (i + 1) * 8 * VP_W],
                vp_d[:, i * 8 * VP_W : (i + 1) * 8 * VP_W],
            )

        # two persistent accumulators (one per q-block), live all pairs
        o_ps = [
            o_pool.tile([128, 512], F32, tag=f"o{q}", name=f"o_ps{q}")
            for q in range(NQ)
        ]
        p_tiles = {}

        def scores_exp(m):
            # kt stationary reused across both q-blocks; chunk m on array
            # rows 0:64 runs row-tile-concurrent with chunk m+32 on 64:128.
            # s tile is chunk-major: cols 0:512 = qblk0, 512:1024 = qblk1.
            # exp: chunk m on ACT (exact), chunk m+32 on DVE (Schraudolph).
            kcol = m * 128
            s_tiles = [
                s_pool.tile([128, 1024], F32, tag="sT", name=f"s{h}") for h in range(2)
            ]
            # q-outer / half-inner so the two row-tiles run truly concurrent:
            # (A q0 || B q0), then (A q1 || B q1)
            for q in range(NQ):
                for h in range(2):
                    rows = slice(64 * h, 64 * h + 64)
                    nc.tensor.matmul(
                        s_tiles[h][:, q * 512 : q * 512 + 512],
                        kt_sb[rows, kcol : kcol + 128],
                        qt_sb[rows, q * 512 : q * 512 + 512],
                        start=True,
                        stop=True,
                    )
            ps = []
            for h in range(2):
                p_sb = p_pool.tile([128, 1024], BF16, tag="pT")
                ps.append(p_sb)
                if h == 0:
                    nc.scalar.activation(p_sb[:], s_tiles[h][:], exp_f, scale=SCALE)
                else:
                    nc.vector.tensor_scalar(
                        p_sb[:].bitcast(I16),
                        s_tiles[h][:],
                        SCH_C1 * SCALE,
                        SCH_C2,
                        op0=mybir.AluOpType.mult,
                        op1=mybir.AluOpType.add,
                    )
            p_tiles[m] = ps

        def pvs(m):
            # vp stationary reused across both q-blocks
            ps = p_tiles.pop(m)
            for h in range(2):
                off = (2 * m + h) * VP_W
                for q in range(NQ):
                    nc.tensor.matmul(
                        o_ps[q][0 : D + 1, :],
                        vp_sb[:, off : off + D + 1],
                        ps[h][:, q * 512 : q * 512 + 512],
                        start=(m == 0 and h == 0),
                        stop=(m == NPAIR - 1 and h == 1),
                        skip_group_check=True,
                    )

        # software pipeline: scores run 2 pairs ahead of PV so the PE queue
        # never heads-of-line blocks on an exp still in flight
        for it in range(NPAIR + 2):
            if it < NPAIR:
                scores_exp(it)
            if it >= 2:
                pvs(it - 2)

        # raw accumulators out; host divides by row 64 and transposes
        for q in range(NQ):
            o_sb = fin_pool.tile([D + 1, 512], F32, tag=f"osb{q}")
            nc.scalar.copy(o_sb[:], o_ps[q][0 : D + 1, :])
            nc.sync.dma_start(acc_d[:, q * 512 : q * 512 + 512], o_sb[:])

    nc.compile()
    return nc


def kernel(x: np.ndarray, w_qkv: np.ndarray) -> np.ndarray:
    global LAST_RESULTS
    LAST_RESULTS = []
    x = np.asarray(x, dtype=np.float32)
    w_qkv = np.asarray(w_qkv, dtype=np.float32)

    if "p1" not in _CACHE:
        _CACHE["p1"] = _build_pass1()
    if "p2" not in _CACHE:
        _CACHE["p2"] = _build_pass2()

    xt = np.ascontiguousarray(x.T).astype(BF_NP)       # [512, 8192]
    wt = np.ascontiguousarray(w_qkv.T)                  # [512, 192]
    # wt SBUF image [128, 4*192]: wt_img[p, i*192+o] = wt[i*128+p, o]
    wt_img = np.ascontiguousarray(
        wt.reshape(4, 128, 3 * D).transpose(1, 0, 2).reshape(128, 4 * 3 * D)
    ).astype(BF_NP)

    in_maps1 = [
        {
            "xt": np.ascontiguousarray(xt[:, c * SEQ_C : (c + 1) * SEQ_C]),
            "wt": wt_img,
        }
        for c in range(NC)
    ]
    res1 = run_bass_kernel_spmd(_CACHE["p1"], in_maps1, core_ids=list(range(NC)))
    LAST_RESULTS.append(res1)

    qk = [res1.results[c]["qk"] for c in range(NC)]            # [128, 1024] bf16
    kt_full = np.concatenate([m[64:128] for m in qk], axis=1)  # [64, 8192]
    vt_full = np.concatenate(
        [res1.results[c]["vt"] for c in range(NC)], axis=1
    )                                                          # [64, 8192]
    v_full = np.ascontiguousarray(vt_full.T)                   # [8192, 64] bf16

    # K^T folded to 128 partitions: rows 0:64 keys 0:4096, rows 64:128 the rest
    kt2 = np.ascontiguousarray(
        np.concatenate([kt_full[:, : N // 2], kt_full[:, N // 2 :]], axis=0)
    )
    # V' image [128, 64*VP_W]: processing position 2m = chunk m, 2m+1 = chunk m+32
    vp = np.zeros((128, (N // 128) * VP_W), dtype=BF_NP)
    for pos in range(64):
        j = (pos // 2) + (pos % 2) * 32
        vp[:, pos * VP_W : pos * VP_W + D] = v_full[j * 128 : (j + 1) * 128, :]
        vp[:, pos * VP_W + D] = 1.0

    in_maps2 = [
        {
            "qt2": np.ascontiguousarray(np.concatenate([qk[c][0:64]] * 2, axis=0)),
            "kt2": kt2,
            "vp": vp,
        }
        for c in range(NC)
    ]
    res2 = run_bass_kernel_spmd(_CACHE["p2"], in_maps2, core_ids=list(range(NC)))
    LAST_RESULTS.append(res2)

    # acc [65, 1024] per core: rows 0:64 = (P V)^T, row 64 = softmax denominator
    out = np.empty((N, D), dtype=np.float32)
    for c in range(NC):
        acc = res2.results[c]["acc"].astype(np.float32)
        out[c * SEQ_C : (c + 1) * SEQ_C, :] = (acc[0:D, :] / acc[D : D + 1, :]).T
    return out
